# revision 1
# baseline (speedup 1.0000x reference)
# Bass/Trainium2 kernel for DSGR message-passing layer (8-core SPMD).
#
# Strategy: shard user/item node axes 8-way. Weights + embedding tables are
# replicated; each core computes the full uh/ih projection tables itself (no
# collectives), stores them to DRAM, and dma_gathers the 50-neighbor mailboxes
# for its 1024+1024 node shard. Time-rank (double-argsort) is computed exactly
# on-device with int32 keys (t*64+l) via a 50x50 compare matrix.
import numpy as np

D = 128
L = 50
NU = 8192
NI = 8192
NCORES = 8
SH = NU // NCORES          # 1024 nodes per core per side
TPC = SH // 128            # 8 tiles per core per side
SCALE = 1.0 / float(np.sqrt(128.0))

_CACHE = {}


def _build_program(sh, nu, ni):
    """Build the (core-uniform) Bass program. sh = shard nodes per side."""
    import os
    STAGE = int(os.environ.get("BASSK_STAGE", "4"))
    MAXT = int(os.environ.get("BASSK_TILES", "99"))
    import concourse.bass as bass
    import concourse.bacc as bacc
    import concourse.mybir as mybir
    import concourse.tile as tile
    from concourse.tile_rust import add_dep_helper

    f32 = mybir.dt.float32
    i32 = mybir.dt.int32
    i16 = mybir.dt.int16
    bf16 = mybir.dt.bfloat16
    Alu = mybir.AluOpType
    Act = mybir.ActivationFunctionType
    AX = mybir.AxisListType

    tpc = sh // 128
    nblk_u = nu // 128
    nblk_i = ni // 128

    nc = bacc.Bacc("TRN2", target_bir_lowering=False, debug=False)

    def inp(name, shape, dtype=f32):
        return nc.declare_dram_parameter(name, list(shape), dtype, isOutput=False)

    # ---- inputs (replicated unless noted) ----
    userT = inp("userT", [D, nu])            # user features, transposed
    itemT = inp("itemT", [D, ni])
    Wu = inp("Wu", [D, D])
    Wi = inp("Wi", [D, D])
    Gut = inp("Gut", [D, D])                 # agg_gate_user rows 0:128
    Gub = inp("Gub", [D, D])                 # agg_gate_user rows 128:256
    Git = inp("Git", [D, D])
    Gib = inp("Gib", [D, D])
    Uut = inp("Uut", [D, D])                 # update_user rows 0:128
    Uub = inp("Uub", [D, D])
    Uit = inp("Uit", [D, D])
    Uib = inp("Uib", [D, D])
    uembT = inp("uembT", [D, L])             # user_date_emb transposed
    uembkT = inp("uembkT", [D, L])
    iembT = inp("iembT", [D, L])
    iembkT = inp("iembkT", [D, L])
    featuT = inp("featuT", [D, sh])          # per-core: user feature shard (transposed)
    featiT = inp("featiT", [D, sh])
    utime = inp("utime", [sh, L], i32)       # per-core shard
    itime = inp("itime", [sh, L], i32)
    unbr = inp("unbr", [sh, L], i32)         # per-core shard (node-major neighbor ids)
    inbr = inp("inbr", [sh, L], i32)
    triU = inp("triU", [128, L, L])          # [l,j]: 1.0 if j>l else 0
    iotaf = inp("iotaf", [128, L])
    iota1kf = inp("iota1kf", [128, L])       # l + 1000.0
    ident = inp("ident", [D, D])             # identity for PE transpose
    identb = inp("identb", [D, D], mybir.dt.bfloat16)

    uout = nc.declare_dram_parameter("uout", [sh, D], f32, isOutput=True)
    iout = nc.declare_dram_parameter("iout", [sh, D], f32, isOutput=True)

    # internal DRAM
    uhd = nc.dram_tensor("uhd", [nu, D], bf16)
    ihd = nc.dram_tensor("ihd", [ni, D], bf16)

    IDXW = L * 128 // 16   # 400 idx columns per tile

    with tile.TileContext(nc) as tc:
        with (
            tc.tile_pool(name="const", bufs=1) as constp,
            tc.tile_pool(name="stage", bufs=4) as stagep,
            tc.tile_pool(name="tstage", bufs=2) as tstagep,
            tc.tile_pool(name="mtile", bufs=2) as mpool,
            tc.tile_pool(name="prod", bufs=2) as prodp,
            tc.tile_pool(name="cmat", bufs=3) as cpool,
            tc.tile_pool(name="small", bufs=2) as smallp,
            tc.tile_pool(name="psum", bufs=2, space="PSUM") as psump,
            tc.tile_pool(name="psmm", bufs=4, space="PSUM") as psmm,
        ):
            # ---------- load constants ----------
            def load_const(src, shape, dtype=f32):
                t = constp.tile(list(shape), dtype, tag=src.name)
                nc.sync.dma_start(t[:], src[:])
                return t

            Wu_s = load_const(Wu, [D, D])
            Wi_s = load_const(Wi, [D, D])
            Gut_s = load_const(Gut, [D, D])
            Gub_s = load_const(Gub, [D, D])
            Git_s = load_const(Git, [D, D])
            Gib_s = load_const(Gib, [D, D])
            Uut_s = load_const(Uut, [D, D])
            Uub_s = load_const(Uub, [D, D])
            Uit_s = load_const(Uit, [D, D])
            Uib_s = load_const(Uib, [D, D])
            uembT_s = load_const(uembT, [D, L])
            uembkT_s = load_const(uembkT, [D, L])
            iembT_s = load_const(iembT, [D, L])
            iembkT_s = load_const(iembkT, [D, L])
            featu_s = load_const(featuT, [D, sh])
            feati_s = load_const(featiT, [D, sh])
            triU_s = load_const(triU, [128, L, L])
            iotaf_s = load_const(iotaf, [128, L])
            iota1kf_s = load_const(iota1kf, [128, L])
            ident_s = load_const(ident, [D, D])
            identb_s = load_const(identb, [D, D], mybir.dt.bfloat16)

            # ---------- build tables uhd/ihd (full, replicated) ----------
            table_dmas = {"u": [], "i": []}
            for side, srcT, W_s, dstd, nblk in (
                ("u", userT, Wu_s, uhd, nblk_u),
                ("i", itemT, Wi_s, ihd, nblk_i),
            ):
                for grp in range(nblk // 4):
                    pt = psump.tile([128, 512], f32, tag="ptab")
                    for j in range(4):
                        b = grp * 4 + j
                        blk = stagep.tile([D, 128], f32, tag="ldblk")
                        nc.sync.dma_start(blk[:], srcT[:, b * 128:(b + 1) * 128])
                        nc.tensor.matmul(
                            pt[:, j * 128:(j + 1) * 128], blk[:], W_s[:],
                            start=True, stop=True,
                        )
                    st = tstagep.tile([128, 512], bf16, tag="tstg")
                    nc.scalar.copy(st[:], pt[:])
                    dmai = nc.sync.dma_start(
                        dstd[grp * 512:(grp + 1) * 512, :].rearrange(
                            "(j p) f -> p j f", p=128
                        ),
                        st[:].rearrange("p (j f) -> p j f", f=128),
                    )
                    table_dmas[side].append(dmai)

            # ---------- per-shard projections ----------
            # uh_sh (node-major) and uhT_sh (d-major) for each side
            def shard_proj(feat_s, W_s, nm):
                h_nm = constp.tile([128, tpc, D], bf16, tag="hnm_" + nm)
                h_T = constp.tile([D, sh], f32, tag="hT_" + nm)
                for t in range(tpc):
                    p1 = psmm.tile([128, D], f32, tag="mm")
                    nc.tensor.matmul(
                        p1[:], feat_s[:, t * 128:(t + 1) * 128], W_s[:],
                        start=True, stop=True,
                    )
                    nc.scalar.copy(h_nm[:, t, :], p1[:])
                for q in range((sh + 511) // 512):
                    w = min(512, sh - q * 512)
                    p2 = psump.tile([128, 512], f32, tag="ptab")
                    nc.tensor.matmul(
                        p2[:, 0:w], W_s[:], feat_s[:, q * 512:q * 512 + w],
                        start=True, stop=True,
                    )
                    nc.scalar.copy(h_T[:, q * 512:q * 512 + w], p2[:, 0:w])
                return h_nm, h_T

            uh_nm, uh_T = shard_proj(featu_s, Wu_s, 'u')
            ih_nm, ih_T = shard_proj(feati_s, Wi_s, 'i')

            # emb_k @ G_top per side  -> [L, D]
            def embk_gate(embkT_s, Gt_s, nm):
                p = psmm.tile([L, D], f32, tag="mm")
                nc.tensor.matmul(p[:], embkT_s[:], Gt_s[:], start=True, stop=True)
                s = constp.tile([L, D], f32, tag="ekg_" + nm)
                nc.scalar.copy(s[:], p[:])
                return s

            embkG_u = embk_gate(uembkT_s, Gut_s, 'u')
            embkG_i = embk_gate(iembkT_s, Git_s, 'i')

            # ---------- main per-tile loop ----------
            sides = (
                (0, utime, unbr, ihd, uh_nm, uh_T, uembT_s, embkG_u,
                 Gut_s, Gub_s, Uut_s, Uub_s, featu_s, uout),
                (1, itime, inbr, uhd, ih_nm, ih_T, iembT_s, embkG_i,
                 Git_s, Gib_s, Uit_s, Uib_s, feati_s, iout),
            )

            for (sidx, timeh, nbrh, tabled, own_nm, own_T, embT_s, embkG_s,
                 Gt_s, Gb_s, Ut_s, Ub_s, featT_s, outh) in sides:
                opp_dmas = table_dmas["u" if sidx == 1 else "i"]
                for t in range(min(tpc, MAXT)):
                    r0 = t * 128
                    # -- loads --
                    time_s = smallp.tile([128, L], i32, tag="time")
                    nc.sync.dma_start(time_s[:], timeh[r0:r0 + 128, :])
                    nbr_s = smallp.tile([128, L], i32, tag="nbr")
                    nc.sync.dma_start(nbr_s[:], nbrh[r0:r0 + 128, :])
                    # -- main mailbox gather (slots 0..49), one slot per call:
                    # HW indirect DMA pairs one offset per contiguous
                    # descriptor, so gather row-by-row ([128,1] offsets).
                    M = mpool.tile([128, L + 1, D], bf16, tag="M")
                    for _l in range(L):
                        g1 = nc.gpsimd.indirect_dma_start(
                            out=M[:, _l, :],
                            out_offset=None,
                            in_=tabled[:, :],
                            in_offset=bass.IndirectOffsetOnAxis(
                                ap=nbr_s[:, _l:_l + 1], axis=0
                            ),
                        )
                        if _l == 0:
                            for dmai in opp_dmas:
                                add_dep_helper(g1.ins, dmai.ins, reason="table RAW")

                    if STAGE == 1:
                        o1 = smallp.tile([128, D], f32, tag="out")
                        nc.vector.tensor_copy(o1[:], M[:, 0, :])
                        nc.sync.dma_start(outh[r0:r0 + 128, :], o1[:])
                        continue

                    # -- last = argmax(time) (first max), then gather M_last --
                    tf = smallp.tile([128, L], f32, tag="tf")
                    nc.vector.tensor_copy(tf[:], time_s[:])
                    tmaxf = smallp.tile([128, 1], f32, tag="tmaxf")
                    nc.vector.reduce_max(tmaxf[:], tf[:], axis=AX.X)
                    maskT = smallp.tile([128, L], f32, tag="maskT")
                    nc.vector.tensor_tensor(
                        maskT[:], tf[:],
                        tmaxf[:].broadcast_to([128, L]), Alu.is_equal,
                    )
                    lv1 = smallp.tile([128, L], f32, tag="lv1")
                    nc.vector.tensor_scalar(
                        lv1[:], maskT[:], -1000.0, None, op0=Alu.mult
                    )
                    lastv = smallp.tile([128, L], f32, tag="lastv")
                    nc.vector.tensor_tensor(lastv[:], lv1[:], iota1kf_s[:], Alu.add)
                    lastlf = smallp.tile([128, 1], f32, tag="lastlf")
                    nc.vector.tensor_reduce(
                        lastlf[:], lastv[:], axis=AX.X, op=Alu.min
                    )
                    maskL = smallp.tile([128, L], f32, tag="maskL")
                    nc.vector.tensor_tensor(
                        maskL[:], iotaf_s[:],
                        lastlf[:].broadcast_to([128, L]), Alu.is_equal,
                    )
                    nbrf = smallp.tile([128, L], f32, tag="nbrf")
                    nc.vector.tensor_copy(nbrf[:], nbr_s[:])
                    pmv = smallp.tile([128, L], f32, tag="pmv")
                    nc.vector.tensor_tensor(pmv[:], nbrf[:], maskL[:], Alu.mult)
                    lastnbr = smallp.tile([128, 1], f32, tag="lastnbr")
                    nc.vector.reduce_sum(lastnbr[:], pmv[:], axis=AX.X)
                    lastn_i = smallp.tile([128, 1], i32, tag="lastni")
                    nc.vector.tensor_copy(lastn_i[:], lastnbr[:])
                    g2 = nc.gpsimd.indirect_dma_start(
                        out=M[:, L, :],
                        out_offset=None,
                        in_=tabled[:, :],
                        in_offset=bass.IndirectOffsetOnAxis(
                            ap=lastn_i[:, :1], axis=0
                        ),
                    )
                    for dmai in opp_dmas:
                        add_dep_helper(g2.ins, dmai.ins, reason="table RAW")

                    if STAGE == 15:
                        o15 = smallp.tile([128, D], f32, tag="out")
                        nc.vector.tensor_copy(o15[:], M[:, L, :])
                        nc.sync.dma_start(outh[r0:r0 + 128, :], o15[:])
                        continue

                    # -- ranks (f32, exact): rank_l = #(t_j > t_l) + #(j>l & t_j==t_l)
                    G1 = cpool.tile([128, L, L], f32, tag="c3")
                    nc.vector.tensor_tensor(
                        G1[:],
                        tf[:].unsqueeze(1).broadcast_to([128, L, L]),
                        tf[:].unsqueeze(2).broadcast_to([128, L, L]),
                        Alu.is_gt,
                    )
                    GE = cpool.tile([128, L, L], f32, tag="c3")
                    nc.vector.tensor_tensor(
                        GE[:],
                        tf[:].unsqueeze(1).broadcast_to([128, L, L]),
                        tf[:].unsqueeze(2).broadcast_to([128, L, L]),
                        Alu.is_ge,
                    )
                    DM = cpool.tile([128, L, L], f32, tag="c3")
                    nc.vector.tensor_tensor(DM[:], GE[:], G1[:], Alu.subtract)
                    nc.vector.tensor_tensor(DM[:], DM[:], triU_s[:], Alu.mult)
                    cmp = G1
                    nc.vector.tensor_tensor(cmp[:], G1[:], DM[:], Alu.add)
                    rank = smallp.tile([128, L], f32, tag="rank")
                    nc.vector.reduce_sum(rank[:], cmp[:], axis=AX.X)

                    if STAGE == 18:
                        o18 = smallp.tile([128, D], f32, tag="out")
                        nc.vector.memset(o18[:], 0.0)
                        nc.vector.tensor_copy(o18[:, 0:L], rank[:])
                        nc.sync.dma_start(outh[r0:r0 + 128, :], o18[:])
                        continue
                    C = cpool.tile([128, L, L], f32, tag="c3")
                    nc.vector.tensor_tensor(
                        C[:],
                        rank[:].unsqueeze(2).broadcast_to([128, L, L]),
                        iotaf_s[:].unsqueeze(1).broadcast_to([128, L, L]),
                        Alu.is_equal,
                    )

                    # -- P = dst_h @ embT  -> [128n, L] --
                    pP = psmm.tile([128, L], f32, tag="mm")
                    nc.tensor.matmul(
                        pP[:], own_T[:, r0:r0 + 128], embT_s[:],
                        start=True, stop=True,
                    )
                    P_s = smallp.tile([128, L], f32, tag="P")
                    nc.scalar.copy(P_s[:], pP[:])

                    # -- e_pos = C @ P (per-row permute) --
                    X2 = cpool.tile([128, L, L], f32, tag="c3")
                    nc.vector.tensor_tensor(
                        X2[:], C[:],
                        P_s[:].unsqueeze(1).broadcast_to([128, L, L]),
                        Alu.mult,
                    )
                    e_pos = smallp.tile([128, L], f32, tag="epos")
                    nc.vector.reduce_sum(e_pos[:], X2[:], axis=AX.X)

                    if STAGE == 2:
                        o2 = smallp.tile([128, D], f32, tag="out")
                        nc.vector.memset(o2[:], 0.0)
                        nc.vector.tensor_copy(o2[:, 0:L], e_pos[:])
                        nc.vector.tensor_copy(o2[:, L:L + 1], lastnbr[:])
                        nc.sync.dma_start(outh[r0:r0 + 128, :], o2[:])
                        continue

                    # -- m_dot = sum_d M * dst_h --
                    prodM = prodp.tile([128, L, D], bf16, tag="prod")
                    nc.vector.tensor_tensor(
                        prodM[:], M[:, 0:L, :],
                        own_nm[:, t, :].unsqueeze(1).broadcast_to([128, L, D]),
                        Alu.mult,
                    )
                    m_dot = smallp.tile([128, L], f32, tag="mdot")
                    nc.vector.reduce_sum(m_dot[:], prodM[:], axis=AX.X)

                    # -- alpha = softmax(scale*(m_dot + e_pos)) --
                    e_t = smallp.tile([128, L], f32, tag="e")
                    nc.vector.tensor_tensor(e_t[:], m_dot[:], e_pos[:], Alu.add)
                    emax = smallp.tile([128, 1], f32, tag="emax")
                    nc.vector.reduce_max(emax[:], e_t[:], axis=AX.X)
                    negb = smallp.tile([128, 1], f32, tag="negb")
                    nc.vector.tensor_scalar(
                        negb[:], emax[:], -SCALE, None, op0=Alu.mult
                    )
                    ex = smallp.tile([128, L], f32, tag="ex")
                    Z = smallp.tile([128, 1], f32, tag="Z")
                    nc.scalar.activation(
                        ex[:], e_t[:], Act.Exp,
                        bias=negb[:], scale=SCALE, accum_out=Z[:],
                    )
                    rZ = smallp.tile([128, 1], f32, tag="rZ")
                    nc.vector.reciprocal(rZ[:], Z[:])
                    alpha = smallp.tile([128, L], f32, tag="alpha")
                    nc.vector.tensor_scalar(
                        alpha[:], ex[:], rZ[:], None, op0=Alu.mult
                    )

                    alphab = smallp.tile([128, L], bf16, tag="alphab")
                    nc.vector.tensor_copy(alphab[:], alpha[:])

                    # -- beta[r] = sum_l alpha_l C[l,r] --
                    X3 = cpool.tile([128, L, L], f32, tag="c3")
                    nc.vector.tensor_tensor(
                        X3[:], C[:].transpose([0, 2, 1]),
                        alpha[:].unsqueeze(1).broadcast_to([128, L, L]),
                        Alu.mult,
                    )
                    beta = smallp.tile([128, L], f32, tag="beta")
                    nc.vector.reduce_sum(beta[:], X3[:], axis=AX.X)

                    # -- h_long1 = sum_l alpha_l M_l --
                    prodA = prodp.tile([128, L, D], bf16, tag="prod")
                    nc.vector.tensor_tensor(
                        prodA[:], M[:, 0:L, :],
                        alphab[:].unsqueeze(2).broadcast_to([128, L, D]),
                        Alu.mult,
                    )
                    ph_l = psmm.tile([128, D], f32, tag="mm")
                    for _l in range(L):
                        nc.tensor.matmul(
                            ph_l[:], identb_s[:], prodA[:, _l, :],
                            start=(_l == 0), stop=(_l == L - 1),
                        )
                    h_long = smallp.tile([128, D], f32, tag="hlong")
                    nc.scalar.copy(h_long[:], ph_l[:])

                    if STAGE == 3:
                        o3 = smallp.tile([128, D], f32, tag="out")
                        nc.vector.tensor_copy(o3[:], h_long[:])
                        nc.sync.dma_start(outh[r0:r0 + 128, :], o3[:])
                        continue

                    # -- e1 / alpha1 / h_short --
                    prodE = prodp.tile([128, L, D], bf16, tag="prod")
                    nc.vector.tensor_tensor(
                        prodE[:], M[:, 0:L, :],
                        M[:, L:L + 1, :].broadcast_to([128, L, D]),
                        Alu.mult,
                    )
                    e1 = smallp.tile([128, L], f32, tag="e1")
                    nc.vector.reduce_sum(e1[:], prodE[:], axis=AX.X)
                    e1max = smallp.tile([128, 1], f32, tag="e1max")
                    nc.vector.reduce_max(e1max[:], e1[:], axis=AX.X)
                    negb1 = smallp.tile([128, 1], f32, tag="negb1")
                    nc.vector.tensor_scalar(
                        negb1[:], e1max[:], -SCALE, None, op0=Alu.mult
                    )
                    ex1 = smallp.tile([128, L], f32, tag="ex1")
                    Z1 = smallp.tile([128, 1], f32, tag="Z1")
                    nc.scalar.activation(
                        ex1[:], e1[:], Act.Exp,
                        bias=negb1[:], scale=SCALE, accum_out=Z1[:],
                    )
                    rZ1 = smallp.tile([128, 1], f32, tag="rZ1")
                    nc.vector.reciprocal(rZ1[:], Z1[:])
                    alpha1 = smallp.tile([128, L], f32, tag="alpha1")
                    nc.vector.tensor_scalar(
                        alpha1[:], ex1[:], rZ1[:], None, op0=Alu.mult
                    )
                    alpha1b = smallp.tile([128, L], bf16, tag="alpha1b")
                    nc.vector.tensor_copy(alpha1b[:], alpha1[:])
                    prodS = prodp.tile([128, L, D], bf16, tag="prod")
                    nc.vector.tensor_tensor(
                        prodS[:], M[:, 0:L, :],
                        alpha1b[:].unsqueeze(2).broadcast_to([128, L, D]),
                        Alu.mult,
                    )
                    ph_s = psmm.tile([128, D], f32, tag="mm")
                    for _l in range(L):
                        nc.tensor.matmul(
                            ph_s[:], identb_s[:], prodS[:, _l, :],
                            start=(_l == 0), stop=(_l == L - 1),
                        )
                    h_short = smallp.tile([128, D], f32, tag="hshort")
                    nc.scalar.copy(h_short[:], ph_s[:])

                    # -- transposes for gate matmul --
                    pT1 = psmm.tile([128, D], f32, tag="mm")
                    nc.tensor.transpose(pT1[:], h_long[:], ident_s[:])
                    hlT = smallp.tile([128, D], f32, tag="hlT")
                    nc.scalar.copy(hlT[:], pT1[:])
                    pT2 = psmm.tile([128, D], f32, tag="mm")
                    nc.tensor.transpose(pT2[:], h_short[:], ident_s[:])
                    hsT = smallp.tile([128, D], f32, tag="hsT")
                    nc.scalar.copy(hsT[:], pT2[:])
                    pT3 = psmm.tile([L, 128], f32, tag="mm")
                    nc.tensor.transpose(pT3[:], beta[:], ident_s[:])
                    betaT = smallp.tile([L, 128], f32, tag="betaT")
                    nc.scalar.copy(betaT[:], pT3[:])

                    # -- gate: g = hl@Gt + hs@Gb + beta@(embk@Gt) --
                    pg = psmm.tile([128, D], f32, tag="mm")
                    nc.tensor.matmul(pg[:], hlT[:], Gt_s[:], start=True, stop=False)
                    nc.tensor.matmul(pg[:], hsT[:], Gb_s[:], start=False, stop=True)
                    pb = psmm.tile([128, D], f32, tag="mm")
                    nc.tensor.matmul(pb[:], betaT[:], embkG_s[:], start=True, stop=True)
                    gb_s = smallp.tile([128, D], f32, tag="gb")
                    nc.scalar.copy(gb_s[:], pb[:])
                    g_s = smallp.tile([128, D], f32, tag="g")
                    nc.vector.tensor_tensor(g_s[:], pg[:], gb_s[:], Alu.add)
                    pT4 = psmm.tile([128, D], f32, tag="mm")
                    nc.tensor.transpose(pT4[:], g_s[:], ident_s[:])
                    gT = smallp.tile([128, D], f32, tag="gT")
                    nc.scalar.copy(gT[:], pT4[:])

                    # -- update: out = tanh(g@Ut + feat@Ub) --
                    po = psmm.tile([128, D], f32, tag="mm")
                    nc.tensor.matmul(po[:], gT[:], Ut_s[:], start=True, stop=False)
                    nc.tensor.matmul(
                        po[:], featT_s[:, r0:r0 + 128], Ub_s[:],
                        start=False, stop=True,
                    )
                    out_s = smallp.tile([128, D], f32, tag="out")
                    nc.scalar.activation(out_s[:], po[:], Act.Tanh)
                    nc.sync.dma_start(outh[r0:r0 + 128, :], out_s[:])

    nc.compile()
    return nc


def _wrap_idxs(nbr_shard):
    """[sh, L] int -> [128, tpc*L*128/16] int16 wrapped for dma_gather."""
    sh = nbr_shard.shape[0]
    tpc = sh // 128
    cols = []
    for t in range(tpc):
        blk = nbr_shard[t * 128:(t + 1) * 128, :]          # [128, L]
        req = blk.T.reshape(-1)                            # i = l*128 + n
        w = req.reshape(-1, 16).T                          # [16, L*128/16]
        cols.append(np.tile(w, (8, 1)))                    # replicate to 128 rows
    return np.concatenate(cols, axis=1).astype(np.int16)


def kernel(**inputs):
    from concourse.bass_utils import run_bass_kernel_spmd

    user = np.asarray(inputs["user"], np.float32)
    item = np.asarray(inputs["item"], np.float32)
    W_user = np.asarray(inputs["W_user"], np.float32)
    W_item = np.asarray(inputs["W_item"], np.float32)
    agu = np.asarray(inputs["agg_gate_user"], np.float32)
    agi = np.asarray(inputs["agg_gate_item"], np.float32)
    upu = np.asarray(inputs["update_user"], np.float32)
    upi = np.asarray(inputs["update_item"], np.float32)
    uemb = np.asarray(inputs["user_date_emb"], np.float32)
    uembk = np.asarray(inputs["user_date_emb_k"], np.float32)
    iemb = np.asarray(inputs["item_date_emb"], np.float32)
    iembk = np.asarray(inputs["item_date_emb_k"], np.float32)
    unbr = np.asarray(inputs["user_nbr"], np.int64).astype(np.int32)
    untime = np.asarray(inputs["user_nbr_time"], np.int64).astype(np.int32)
    inbr = np.asarray(inputs["item_nbr"], np.int64).astype(np.int32)
    intime = np.asarray(inputs["item_nbr_time"], np.int64).astype(np.int32)

    nu, d = user.shape
    ni = item.shape[0]
    sh = nu // NCORES

    key = (sh, nu, ni)
    if key not in _CACHE:
        _CACHE[key] = _build_program(sh, nu, ni)
    nc = _CACHE[key]

    iota = np.arange(L, dtype=np.int32)
    iotaf = np.tile(iota, (128, 1)).astype(np.float32)
    iota1kf = iotaf + 1000.0
    tri = (iota[None, :] > iota[:, None]).astype(np.float32)   # [l, j]: j > l
    triU = np.broadcast_to(tri, (128, L, L)).copy()
    ident = np.eye(D, dtype=np.float32)

    common = dict(
        userT=np.ascontiguousarray(user.T),
        itemT=np.ascontiguousarray(item.T),
        Wu=W_user, Wi=W_item,
        Gut=np.ascontiguousarray(agu[:D]), Gub=np.ascontiguousarray(agu[D:]),
        Git=np.ascontiguousarray(agi[:D]), Gib=np.ascontiguousarray(agi[D:]),
        Uut=np.ascontiguousarray(upu[:D]), Uub=np.ascontiguousarray(upu[D:]),
        Uit=np.ascontiguousarray(upi[:D]), Uib=np.ascontiguousarray(upi[D:]),
        uembT=np.ascontiguousarray(uemb.T), uembkT=np.ascontiguousarray(uembk.T),
        iembT=np.ascontiguousarray(iemb.T), iembkT=np.ascontiguousarray(iembk.T),
        triU=triU, iotaf=iotaf, iota1kf=iota1kf, ident=ident,
        identb=ident.astype(__import__('ml_dtypes').bfloat16),
    )

    in_maps = []
    for c in range(NCORES):
        su = slice(c * sh, (c + 1) * sh)
        m = dict(common)
        m["featuT"] = np.ascontiguousarray(user.T[:, su])
        m["featiT"] = np.ascontiguousarray(item.T[:, su])
        m["utime"] = np.ascontiguousarray(untime[su])
        m["itime"] = np.ascontiguousarray(intime[su])
        m["unbr"] = np.ascontiguousarray(unbr[su])
        m["inbr"] = np.ascontiguousarray(inbr[su])
        in_maps.append(m)

    _LAST_RUN["nc"] = nc
    _LAST_RUN["in_maps"] = in_maps
    res = run_bass_kernel_spmd(nc, in_maps, list(range(NCORES)))
    user_out = np.concatenate([res.results[c]["uout"] for c in range(NCORES)], 0)
    item_out = np.concatenate([res.results[c]["iout"] for c in range(NCORES)], 0)
    return user_out, item_out


_LAST_RUN = {}


def _install_ntff_hook():
    import sys as _sys
    import types as _types
    try:
        from antenv.axon_hooks import get_axon_ntff_profile_hook  # noqa: F401
        return
    except ImportError:
        pass
    if "/root/.axon_site" not in _sys.path:
        _sys.path.insert(0, "/root/.axon_site")
    from trn_agent_boot.trn_boot import _ntff_profile_via_ctypes
    hook = _ntff_profile_via_ctypes("/opt/axon/libaxon_pjrt.so")
    mod = _types.ModuleType("antenv.axon_hooks")
    mod.get_axon_ntff_profile_hook = lambda: hook
    mod.set_axon_ntff_profile_hook = lambda h: None
    _sys.modules["antenv.axon_hooks"] = mod
    import antenv
    antenv.axon_hooks = mod


def bench(n=1):
    """Re-run the cached program with trace=True and return HW exec ns."""
    import time as _t
    from concourse.bass_utils import run_bass_kernel_spmd
    nc = _LAST_RUN["nc"]
    in_maps = _LAST_RUN["in_maps"]
    best = None
    try:
        _install_ntff_hook()
        for _ in range(n):
            res = run_bass_kernel_spmd(
                nc, in_maps, list(range(NCORES)), trace=True
            )
            t = res.exec_time_ns or res.mean_exec_time_ns
            if t and (best is None or t < best):
                best = t
    except Exception as e:
        print("bench trace path failed:", repr(e))
    if best is None:
        # fallback: wall-clock around the execute (includes dispatch)
        for _ in range(3):
            t0 = _t.perf_counter()
            run_bass_kernel_spmd(nc, in_maps, list(range(NCORES)))
            dt = (_t.perf_counter() - t0) * 1e9
            best = dt if best is None or dt < best else best
    return int(best)

